# revision 16
# baseline (speedup 1.0000x reference)
"""AdaptiveECE on 8 Trainium2 NeuronCores.

Data-parallel over N=1,000,000 rows: each core streams its 125,000-row
shard of logits [N,128] through SBUF exactly once (the memory-bound part,
64MB/core at ~358GB/s/core HBM) and reduces it to two small per-row
tensors:

  - mt[r] = max_c x[r,c]           (bit-exact fp32 max)
  - s[r]  = sum_c exp(x[r,c])

v3 engine split: the baseline balanced all reduce work across VectorE +
ScalarE (both ~226us busy vs a ~196us DMA stream) because the stock DVE
tensor_reduce streams only 1 element/cycle/lane. v3 replaces it with two
hand-built custom-DVE micro-op programs (per-NEFF opcode_table rows via
concourse's custom-DVE infra):

  SEG_MAX_PAIR / SEG_SUM_PAIR: a 3-state segmented scan. in0/in1 stream
  the two halves of each row (2 elements/cycle through both SBUF read
  ports), the body computes max(a,b) (resp. a+b), a scan stage folds it
  with same-stage CURR_ALU_OUT feedback, a STEP state entered on
  SUB_DIM_DONE resets the accumulator at row boundaries (BYPASS of the
  fresh element), and out_last_subdim_enable writes one result per row.
  One instruction per chunk computes every row's full 128-wide reduce at
  2 elem/cycle - exactly the uop program AWS never wrote for
  tensor_reduce.

Per-core busy time: DMA ~193us (the stream, now the sole bottleneck),
VectorE ~137us (both segmented reduces), ScalarE ~110us (whole-chunk Exp
big-ops), TensorE/GpSimd idle. The max stays bit-exact fp32, so the host
accuracy test (logits[r, labels[r]] == mt[r]) is unchanged.

The host finishes with O(N) work as the problem's sharding hint
prescribes ("finish ECE on one host"): conf = exp(mt)/s, accuracy, the
global sort of confidences, equal-count bin edges via interp,
searchsorted binning, per-bin (count, conf_sum, acc_sum), and the ECE.

Layout: each partition line holds G=8 consecutive rows (4KB contiguous DMA
runs). Output column (t*G + j), partition p  <->  shard row t*G*128 + p*G + j.
"""

import copy
import sys
import types
from contextlib import ExitStack

import numpy as np

import concourse.bass as bass
import concourse.tile as tile
from concourse import bacc, mybir
from concourse.bass_utils import run_bass_kernel_spmd
import concourse.dve_ops as dve_ops
from concourse.dve_ops import DveOp, OPS
from concourse.dve_spec import Spec, Src0, Src1, AluOp, scan, lower, maxx
from concourse.dve_uop import DveOpSpec, Trigger, AluInp


def _ensure_ntff_hook():
    """bass_utils imports antenv.axon_hooks when tracing is requested; the
    agent image lacks that module. Recreate it (wired to the axon .so) so a
    stray BASS_TRACE=1 in the environment cannot crash the run."""
    try:
        import antenv.axon_hooks  # noqa: F401
        return
    except ImportError:
        pass
    try:
        import antenv
        import trn_agent_boot.trn_boot as tb

        mod = types.ModuleType("antenv.axon_hooks")
        holder = [None]
        mod.set_axon_ntff_profile_hook = lambda h: holder.__setitem__(0, h)
        mod.get_axon_ntff_profile_hook = lambda: holder[0]
        sys.modules["antenv.axon_hooks"] = mod
        antenv.axon_hooks = mod
        try:
            mod.set_axon_ntff_profile_hook(
                tb._ntff_profile_via_ctypes("/opt/axon/libaxon_pjrt.so")
            )
        except Exception:
            pass
    except Exception:
        pass


_ensure_ntff_hook()


# --- custom segmented-reduce DVE ops --------------------------------------

class _SegPairOp(DveOp):
    """Hand-edited custom-DVE op: segmented 2-stream reduce.

    lower() on the (legal) body `scan(op, pair(Src0, Src1))` yields a 2-state
    [seed, steady] program whose scan stage folds with CURR_ALU_OUT feedback
    but never resets. This subclass clones steady into a STEP state that
    BYPASSes the fresh element into the accumulator (= reset to identity,
    inclusive), wires SUB_DIM_DONE transitions like the stock paged-mask
    3-state machine, and write-gates output to the last element of each
    subdim. Result: out[p, s] = reduce over the 2*N elements of row s, at 2
    elements/cycle."""

    def __init__(self, name, spec):
        object.__setattr__(self, "_hand_cache", {})
        super().__init__(name, spec, subdim=True, uops_sha={})

    def compile(self, ver):
        if ver in self._hand_cache:
            return self._hand_cache[ver]
        base = lower(self.spec, ver=ver)
        assert len(base) == 2, f"expected [seed, steady], got {len(base)}"
        seed, steady = copy.deepcopy(base[0]), copy.deepcopy(base[1])
        scan_st = [i for i, dp in enumerate(steady.datapath_config)
                   if dp.alu_src0 == AluInp.CURR_ALU_OUT]
        assert len(scan_st) == 1, scan_st
        step = copy.deepcopy(steady)
        dp = step.datapath_config[scan_st[0]]
        dp.op = AluOp.BYPASS
        dp.alu_src0 = AluInp.PREV_ALU_OUT
        dp.alu_src1 = AluInp.PREV_ALU_OUT
        steady.trigger = (Trigger.SRC_TENSOR_DONE, Trigger.SUB_DIM_DONE, Trigger.NONE)
        steady.next_uop = (0, 2, 0)
        step.trigger = (Trigger.SRC_TENSOR_DONE, Trigger.SUB_DIM_DONE, Trigger.COUNT)
        step.next_uop = (0, 2, 1)
        step.repeat_count = 1
        for u in (steady, step):
            u.out_last_subdim_enable = 1
        r = DveOpSpec(
            name=self.name,
            opcode=dve_ops.get_dve_sub_opcode(self.name),
            uops=[seed, steady, step],
            rd1_en=True,
        )
        self._hand_cache[ver] = r
        return r


def _register(op):
    if op.name in dve_ops._SUB_OPCODE_FOR_NAME:
        return next(o for o in OPS if o.name == op.name)
    OPS.append(op)
    dve_ops._SUB_OPCODE_FOR_NAME[op.name] = (
        dve_ops._CUSTOM_DVE_ROW_BASE + len(OPS) - 1
    )
    dve_ops.CUSTOM_DVE_SPECS[op.name] = op.spec
    return op


def _ref_seg_max(in0, in1, s0, s1, imm2):
    return np.maximum(in0, in1).max(axis=-1)


def _ref_seg_sum(in0, in1, s0, s1, imm2):
    return (in0.astype(np.float32) + in1).sum(axis=-1, dtype=np.float32)


SEG_MAX_PAIR = _register(_SegPairOp(
    "SEG_MAX_PAIR",
    Spec(body=scan(AluOp.MAX, maxx(Src0, Src1)), reference=_ref_seg_max),
))
SEG_SUM_PAIR = _register(_SegPairOp(
    "SEG_SUM_PAIR",
    Spec(body=scan(AluOp.ADD, Src0 + Src1), reference=_ref_seg_sum),
))


# --- kernel ----------------------------------------------------------------

N = 1_000_000
C = 128
NBINS = 15
NCORES = 8
ROWS = N // NCORES  # 125_000 per core
G = 8  # rows per partition line (4KB contiguous DMA runs)
CHUNK_T = 8  # t-groups (of G*128 rows) per chunk

_CACHE: dict = {}
LAST_RESULT = None  # BassKernelResults of the most recent device run


def _build(rows: int, chunk_t: int = CHUNK_T):
    gr = G * 128  # rows per t-group
    tfull = rows // gr  # full t-groups
    tail = rows - tfull * gr  # leftover rows
    tail_p = tail // G  # tail partitions (tail must divide by G)
    assert tail % G == 0, (rows, tail)
    tt = (tfull + (1 if tail else 0)) * G  # output columns

    nc = bacc.Bacc("TRN2", target_bir_lowering=False, debug=False)
    lg = nc.dram_tensor("logits", [rows, C], mybir.dt.float32, kind="ExternalInput").ap()
    s_d = nc.dram_tensor("s", [128, tt], mybir.dt.float32, kind="ExternalOutput").ap()
    mt_d = nc.dram_tensor("mt", [128, tt], mybir.dt.float32, kind="ExternalOutput").ap()

    # [p, t, (j c)] view: row t*1024 + p*8 + j; (j c) is 4KB-contiguous per (p,t)
    lg_t = (
        lg[0 : tfull * gr, :].rearrange("(t p j) c -> p t (j c)", p=128, j=G)
        if tfull
        else None
    )

    h = C // 2  # pair-fold width: in0/in1 stream the two row halves

    with tile.TileContext(nc) as tc, ExitStack() as ctx:
        singles = ctx.enter_context(tc.tile_pool(name="singles", bufs=1))
        xpool = ctx.enter_context(tc.tile_pool(name="x", bufs=3))
        epool = ctx.enter_context(tc.tile_pool(name="e", bufs=2))

        sraw = singles.tile([128, tt], mybir.dt.float32)
        mt_sb = singles.tile([128, tt], mybir.dt.float32)

        # schedule: tail chunk first (its memset off the drain path), a small
        # ramp-in chunk, big chunks, then tapered chunks to shorten the drain
        chunks = []
        t0 = 0
        while t0 < tfull:
            left = tfull - t0
            if left > 6:
                n = chunk_t if left > chunk_t else left - 2
            elif left > 4:
                n = left - 2
            else:
                n = left
            chunks.append([t0, n, False])
            t0 += n
        if tail:
            chunks.insert(0, [tfull, 0, True])

        for t0, nfull, has_tail in chunks:
            nt = nfull + (1 if has_tail else 0)
            ncols = nt * G  # output columns this chunk
            x = xpool.tile([128, ncols, C], mybir.dt.float32)
            dma_halves = ((0, nfull // 2), (nfull // 2, nfull)) if nfull >= 4 else ((0, nfull),)
            for h0, h1 in dma_halves:
                if h1 > h0:
                    nc.sync.dma_start(
                        x[:, h0 * G : h1 * G, :].rearrange(
                            "p a c -> p (a c)"
                        ).rearrange("p (t b) -> p t b", b=G * C),
                        lg_t[:, t0 + h0 : t0 + h1, :],
                    )
            if has_tail:
                nc.vector.memset(x[:, nfull * G :, :], 0.0)
                tail_src = lg[tfull * gr : rows, :].rearrange("(p j) c -> p (j c)", j=G)
                nc.sync.dma_start(
                    x[0:tail_p, nfull * G :, :].rearrange("p a c -> p (a c)"), tail_src
                )

            lo, hi = t0 * G, t0 * G + ncols

            # one VectorE instruction per reduce: 2 elements/cycle, one
            # result per row via the segmented-scan STEP reset. Compute is
            # emitted per DMA half so the first half's ops overlap the second
            # half's transfer and the x buffer frees right after the last
            # half's (short) ops, keeping the input stream fed.
            e = epool.tile([128, ncols, C], mybir.dt.float32)
            bounds = [0, (nfull // 2) * G, ncols] if nfull > 1 else [0, ncols]
            for c0, c1 in zip(bounds[:-1], bounds[1:]):
                if c1 <= c0:
                    continue
                nc.vector._custom_dve(
                    SEG_MAX_PAIR, out=mt_sb[:, lo + c0 : lo + c1],
                    in0=x[:, c0:c1, 0:h], in1=x[:, c0:c1, h:C],
                )
                nc.scalar.activation(
                    e[:, c0:c1, :], x[:, c0:c1, :],
                    mybir.ActivationFunctionType.Exp,
                )
                nc.vector._custom_dve(
                    SEG_SUM_PAIR, out=sraw[:, lo + c0 : lo + c1],
                    in0=e[:, c0:c1, 0:h], in1=e[:, c0:c1, h:C],
                )

            # stream this chunk's outputs out on the (otherwise idle) gpsimd
            # SWDGE ring: output DMAs wait on the VE reduce chain, and on the
            # input ring that wait head-of-line-blocks the next chunk's input
            # stream (HWDGE rings are FIFO per issuing engine)
            nc.gpsimd.dma_start(s_d[:, lo:hi], sraw[:, lo:hi])
            nc.gpsimd.dma_start(mt_d[:, lo:hi], mt_sb[:, lo:hi])

    nc.compile()
    return nc


def _unpermute(a_2d, rows):
    """Device output [128, TT] -> per-row vector [rows].

    Column t*G+j, partition p <-> row t*G*128 + p*G + j.
    """
    gr = G * 128
    tfull = rows // gr
    tail = rows - tfull * gr
    tail_p = tail // G
    out = np.empty(rows, a_2d.dtype)
    nmain = tfull * gr
    out[:nmain] = (
        a_2d[:, : tfull * G].reshape(128, tfull, G).transpose(1, 0, 2).reshape(-1)
    )
    if tail:
        out[nmain:] = a_2d[:tail_p, tfull * G :].reshape(-1)
    return out


def _finish(conf, acc):
    """Mirror of the reference ECE finishing on host."""
    n = conf.shape[0]
    sorted_conf = np.sort(conf)
    q = np.linspace(0.0, float(n), NBINS + 1, dtype=np.float32)
    edges = np.interp(q, np.arange(n, dtype=np.float32), sorted_conf).astype(np.float32)
    idx = np.searchsorted(edges[1:-1], conf, side="left")
    valid = (conf > edges[0]) & (conf <= edges[-1])
    idx = np.where(valid, idx, NBINS)
    cnt = np.bincount(idx, minlength=NBINS + 1)[:NBINS].astype(np.float32)
    csum = np.bincount(idx, weights=conf.astype(np.float64), minlength=NBINS + 1)[
        :NBINS
    ].astype(np.float32)
    asum = np.bincount(idx, weights=acc.astype(np.float64), minlength=NBINS + 1)[
        :NBINS
    ].astype(np.float32)
    prop = cnt / np.float32(n)
    safe = np.maximum(cnt, 1.0)
    gap = np.abs(csum / safe - asum / safe)
    ece = np.sum(np.where(cnt > 0, gap * prop, 0.0), dtype=np.float32)
    return np.asarray(ece, dtype=np.float32).reshape(1)


def kernel(logits, labels, trace: bool = False):
    global LAST_RESULT
    logits = np.asarray(logits)
    labels = np.asarray(labels)
    assert logits.shape == (N, C), logits.shape

    if "nc" not in _CACHE:
        _CACHE["nc"] = _build(ROWS)
    nc = _CACHE["nc"]

    in_maps = [
        {"logits": np.ascontiguousarray(logits[i * ROWS : (i + 1) * ROWS], np.float32)}
        for i in range(NCORES)
    ]
    res = run_bass_kernel_spmd(nc, in_maps, core_ids=list(range(NCORES)), trace=trace)
    LAST_RESULT = res

    s = np.empty(N, np.float32)
    mt = np.empty(N, np.float32)
    for i in range(NCORES):
        s[i * ROWS : (i + 1) * ROWS] = _unpermute(res.results[i]["s"], ROWS)
        mt[i * ROWS : (i + 1) * ROWS] = _unpermute(res.results[i]["mt"], ROWS)

    # mt = exact per-row max (f32); accuracy = logit at the label equals it
    xlab = logits[np.arange(N), labels.astype(np.int64)]
    acc = (xlab == mt).astype(np.float32)
    conf = (np.exp(mt) / s).astype(np.float32)
    return _finish(conf, acc)


# revision 18
# speedup vs baseline: 1.1484x; 1.1484x over previous
"""AdaptiveECE on 8 Trainium2 NeuronCores.

Data-parallel over N=1,000,000 rows: each core streams its 125,000-row
shard of logits [N,128] through SBUF exactly once (the memory-bound part,
64MB/core at ~358GB/s/core HBM) and reduces it to two small per-row
tensors:

  - mt[r] = max_c x[r,c]           (bit-exact fp32 max)
  - s[r]  = sum_c exp(x[r,c])

v3 engine split: the baseline balanced all reduce work across VectorE +
ScalarE (both ~226us busy vs a ~196us DMA stream) because the stock DVE
tensor_reduce streams only 1 element/cycle/lane. v3 replaces it with two
hand-built custom-DVE micro-op programs (per-NEFF opcode_table rows via
concourse's custom-DVE infra):

  SEG_MAX_PAIR / SEG_SUM_PAIR: a 3-state segmented scan. in0/in1 stream
  the two halves of each row (2 elements/cycle through both SBUF read
  ports), the body computes max(a,b) (resp. a+b), a scan stage folds it
  with same-stage CURR_ALU_OUT feedback, a STEP state entered on
  SUB_DIM_DONE resets the accumulator at row boundaries (BYPASS of the
  fresh element), and out_last_subdim_enable writes one result per row.
  One instruction per chunk computes every row's full 128-wide reduce at
  2 elem/cycle - exactly the uop program AWS never wrote for
  tensor_reduce.

Per-core busy time: DMA ~193us (the stream, now the sole bottleneck),
VectorE ~137us (both segmented reduces), ScalarE ~110us (whole-chunk Exp
big-ops), TensorE/GpSimd idle. The max stays bit-exact fp32, so the host
accuracy test (logits[r, labels[r]] == mt[r]) is unchanged.

The host finishes with O(N) work as the problem's sharding hint
prescribes ("finish ECE on one host"): conf = exp(mt)/s, accuracy, the
global sort of confidences, equal-count bin edges via interp,
searchsorted binning, per-bin (count, conf_sum, acc_sum), and the ECE.

Layout: each partition line holds G=8 consecutive rows (4KB contiguous DMA
runs). Output column (t*G + j), partition p  <->  shard row t*G*128 + p*G + j.
"""

import copy
import sys
import types
from contextlib import ExitStack

import numpy as np

import concourse.bass as bass
import concourse.tile as tile
from concourse import bacc, mybir
from concourse.bass_utils import run_bass_kernel_spmd
import concourse.dve_ops as dve_ops
from concourse.dve_ops import DveOp, OPS
from concourse.dve_spec import Spec, Src0, Src1, AluOp, scan, lower, maxx
from concourse.dve_uop import DveOpSpec, Trigger, AluInp


def _ensure_ntff_hook():
    """bass_utils imports antenv.axon_hooks when tracing is requested; the
    agent image lacks that module. Recreate it (wired to the axon .so) so a
    stray BASS_TRACE=1 in the environment cannot crash the run."""
    try:
        import antenv.axon_hooks  # noqa: F401
        return
    except ImportError:
        pass
    try:
        import antenv
        import trn_agent_boot.trn_boot as tb

        mod = types.ModuleType("antenv.axon_hooks")
        holder = [None]
        mod.set_axon_ntff_profile_hook = lambda h: holder.__setitem__(0, h)
        mod.get_axon_ntff_profile_hook = lambda: holder[0]
        sys.modules["antenv.axon_hooks"] = mod
        antenv.axon_hooks = mod
        try:
            mod.set_axon_ntff_profile_hook(
                tb._ntff_profile_via_ctypes("/opt/axon/libaxon_pjrt.so")
            )
        except Exception:
            pass
    except Exception:
        pass


_ensure_ntff_hook()


# --- custom segmented-reduce DVE ops --------------------------------------

class _SegPairOp(DveOp):
    """Hand-edited custom-DVE op: segmented 2-stream reduce.

    lower() on the (legal) body `scan(op, pair(Src0, Src1))` yields a 2-state
    [seed, steady] program whose scan stage folds with CURR_ALU_OUT feedback
    but never resets. This subclass clones steady into a STEP state that
    BYPASSes the fresh element into the accumulator (= reset to identity,
    inclusive), wires SUB_DIM_DONE transitions like the stock paged-mask
    3-state machine, and write-gates output to the last element of each
    subdim. Result: out[p, s] = reduce over the 2*N elements of row s, at 2
    elements/cycle."""

    def __init__(self, name, spec):
        object.__setattr__(self, "_hand_cache", {})
        super().__init__(name, spec, subdim=True, uops_sha={})

    def compile(self, ver):
        if ver in self._hand_cache:
            return self._hand_cache[ver]
        base = lower(self.spec, ver=ver)
        assert len(base) == 2, f"expected [seed, steady], got {len(base)}"
        seed, steady = copy.deepcopy(base[0]), copy.deepcopy(base[1])
        scan_st = [i for i, dp in enumerate(steady.datapath_config)
                   if dp.alu_src0 == AluInp.CURR_ALU_OUT]
        assert len(scan_st) == 1, scan_st
        step = copy.deepcopy(steady)
        dp = step.datapath_config[scan_st[0]]
        dp.op = AluOp.BYPASS
        dp.alu_src0 = AluInp.PREV_ALU_OUT
        dp.alu_src1 = AluInp.PREV_ALU_OUT
        steady.trigger = (Trigger.SRC_TENSOR_DONE, Trigger.SUB_DIM_DONE, Trigger.NONE)
        steady.next_uop = (0, 2, 0)
        step.trigger = (Trigger.SRC_TENSOR_DONE, Trigger.SUB_DIM_DONE, Trigger.COUNT)
        step.next_uop = (0, 2, 1)
        step.repeat_count = 1
        for u in (steady, step):
            u.out_last_subdim_enable = 1
        r = DveOpSpec(
            name=self.name,
            opcode=dve_ops.get_dve_sub_opcode(self.name),
            uops=[seed, steady, step],
            rd1_en=True,
        )
        self._hand_cache[ver] = r
        return r


def _register(op):
    if op.name in dve_ops._SUB_OPCODE_FOR_NAME:
        return next(o for o in OPS if o.name == op.name)
    OPS.append(op)
    dve_ops._SUB_OPCODE_FOR_NAME[op.name] = (
        dve_ops._CUSTOM_DVE_ROW_BASE + len(OPS) - 1
    )
    dve_ops.CUSTOM_DVE_SPECS[op.name] = op.spec
    return op


def _ref_seg_max(in0, in1, s0, s1, imm2):
    return np.maximum(in0, in1).max(axis=-1)


def _ref_seg_sum(in0, in1, s0, s1, imm2):
    return (in0.astype(np.float32) + in1).sum(axis=-1, dtype=np.float32)


SEG_MAX_PAIR = _register(_SegPairOp(
    "SEG_MAX_PAIR",
    Spec(body=scan(AluOp.MAX, maxx(Src0, Src1)), reference=_ref_seg_max),
))
SEG_SUM_PAIR = _register(_SegPairOp(
    "SEG_SUM_PAIR",
    Spec(body=scan(AluOp.ADD, Src0 + Src1), reference=_ref_seg_sum),
))


# --- kernel ----------------------------------------------------------------

N = 1_000_000
C = 128
NBINS = 15
NCORES = 8
ROWS = N // NCORES  # 125_000 per core
G = 8  # rows per partition line (4KB contiguous DMA runs)
CHUNK_T = 8  # t-groups (of G*128 rows) per chunk

_CACHE: dict = {}
LAST_RESULT = None  # BassKernelResults of the most recent device run


def _build(rows: int, chunk_t: int = CHUNK_T):
    gr = G * 128  # rows per t-group
    tfull = rows // gr  # full t-groups
    tail = rows - tfull * gr  # leftover rows
    tail_p = tail // G  # tail partitions (tail must divide by G)
    assert tail % G == 0, (rows, tail)
    tt = (tfull + (1 if tail else 0)) * G  # output columns

    nc = bacc.Bacc("TRN2", target_bir_lowering=False, debug=False)
    lg = nc.dram_tensor("logits", [rows, C], mybir.dt.float32, kind="ExternalInput").ap()
    s_d = nc.dram_tensor("s", [128, tt], mybir.dt.float32, kind="ExternalOutput").ap()
    mt_d = nc.dram_tensor("mt", [128, tt], mybir.dt.float32, kind="ExternalOutput").ap()

    # [p, t, (j c)] view: row t*1024 + p*8 + j; (j c) is 4KB-contiguous per (p,t)
    lg_t = (
        lg[0 : tfull * gr, :].rearrange("(t p j) c -> p t (j c)", p=128, j=G)
        if tfull
        else None
    )

    h = C // 2  # pair-fold width: in0/in1 stream the two row halves

    with tile.TileContext(nc) as tc, ExitStack() as ctx:
        singles = ctx.enter_context(tc.tile_pool(name="singles", bufs=1))
        xpool = ctx.enter_context(tc.tile_pool(name="x", bufs=3))
        epool = ctx.enter_context(tc.tile_pool(name="e", bufs=2))

        sraw = singles.tile([128, tt], mybir.dt.float32)
        mt_sb = singles.tile([128, tt], mybir.dt.float32)

        # schedule: tail chunk first (its memset off the drain path), a small
        # ramp-in chunk, big chunks, then tapered chunks to shorten the drain
        chunks = []
        t0 = 0
        first = True
        while t0 < tfull:
            left = tfull - t0
            if first:
                n = min(4, left)
                first = False
            elif left > 6:
                n = chunk_t if left > chunk_t else left - 2
            elif left > 4:
                n = left - 2
            else:
                n = left
            chunks.append([t0, n, False])
            t0 += n
        if tail:
            chunks.insert(0, [tfull, 0, True])

        for t0, nfull, has_tail in chunks:
            nt = nfull + (1 if has_tail else 0)
            ncols = nt * G  # output columns this chunk
            x = xpool.tile([128, ncols, C], mybir.dt.float32)
            for h0, h1 in ((0, nfull // 2), (nfull // 2, nfull)):
                if h1 > h0:
                    nc.sync.dma_start(
                        x[:, h0 * G : h1 * G, :].rearrange(
                            "p a c -> p (a c)"
                        ).rearrange("p (t b) -> p t b", b=G * C),
                        lg_t[:, t0 + h0 : t0 + h1, :],
                    )
            if has_tail:
                nc.vector.memset(x[:, nfull * G :, :], 0.0)
                tail_src = lg[tfull * gr : rows, :].rearrange("(p j) c -> p (j c)", j=G)
                nc.sync.dma_start(
                    x[0:tail_p, nfull * G :, :].rearrange("p a c -> p (a c)"), tail_src
                )

            lo, hi = t0 * G, t0 * G + ncols

            # one VectorE instruction per reduce: 2 elements/cycle, one
            # result per row via the segmented-scan STEP reset. Compute is
            # emitted per DMA half so the first half's ops overlap the second
            # half's transfer and the x buffer frees right after the last
            # half's (short) ops, keeping the input stream fed.
            e = epool.tile([128, ncols, C], mybir.dt.float32)
            bounds = [0, (nfull // 2) * G, ncols] if nfull > 1 else [0, ncols]
            for c0, c1 in zip(bounds[:-1], bounds[1:]):
                if c1 <= c0:
                    continue
                nc.vector._custom_dve(
                    SEG_MAX_PAIR, out=mt_sb[:, lo + c0 : lo + c1],
                    in0=x[:, c0:c1, 0:h], in1=x[:, c0:c1, h:C],
                )
                nc.scalar.activation(
                    e[:, c0:c1, :], x[:, c0:c1, :],
                    mybir.ActivationFunctionType.Exp,
                )
                nc.vector._custom_dve(
                    SEG_SUM_PAIR, out=sraw[:, lo + c0 : lo + c1],
                    in0=e[:, c0:c1, 0:h], in1=e[:, c0:c1, h:C],
                )

            # stream this chunk's outputs out on the (otherwise idle) gpsimd
            # SWDGE ring: output DMAs wait on the VE reduce chain, and on the
            # input ring that wait head-of-line-blocks the next chunk's input
            # stream (HWDGE rings are FIFO per issuing engine)
            nc.gpsimd.dma_start(s_d[:, lo:hi], sraw[:, lo:hi])
            nc.gpsimd.dma_start(mt_d[:, lo:hi], mt_sb[:, lo:hi])

    nc.compile()
    return nc


def _unpermute(a_2d, rows):
    """Device output [128, TT] -> per-row vector [rows].

    Column t*G+j, partition p <-> row t*G*128 + p*G + j.
    """
    gr = G * 128
    tfull = rows // gr
    tail = rows - tfull * gr
    tail_p = tail // G
    out = np.empty(rows, a_2d.dtype)
    nmain = tfull * gr
    out[:nmain] = (
        a_2d[:, : tfull * G].reshape(128, tfull, G).transpose(1, 0, 2).reshape(-1)
    )
    if tail:
        out[nmain:] = a_2d[:tail_p, tfull * G :].reshape(-1)
    return out


def _finish(conf, acc):
    """Mirror of the reference ECE finishing on host."""
    n = conf.shape[0]
    sorted_conf = np.sort(conf)
    q = np.linspace(0.0, float(n), NBINS + 1, dtype=np.float32)
    edges = np.interp(q, np.arange(n, dtype=np.float32), sorted_conf).astype(np.float32)
    idx = np.searchsorted(edges[1:-1], conf, side="left")
    valid = (conf > edges[0]) & (conf <= edges[-1])
    idx = np.where(valid, idx, NBINS)
    cnt = np.bincount(idx, minlength=NBINS + 1)[:NBINS].astype(np.float32)
    csum = np.bincount(idx, weights=conf.astype(np.float64), minlength=NBINS + 1)[
        :NBINS
    ].astype(np.float32)
    asum = np.bincount(idx, weights=acc.astype(np.float64), minlength=NBINS + 1)[
        :NBINS
    ].astype(np.float32)
    prop = cnt / np.float32(n)
    safe = np.maximum(cnt, 1.0)
    gap = np.abs(csum / safe - asum / safe)
    ece = np.sum(np.where(cnt > 0, gap * prop, 0.0), dtype=np.float32)
    return np.asarray(ece, dtype=np.float32).reshape(1)


def kernel(logits, labels, trace: bool = False):
    global LAST_RESULT
    logits = np.asarray(logits)
    labels = np.asarray(labels)
    assert logits.shape == (N, C), logits.shape

    if "nc" not in _CACHE:
        _CACHE["nc"] = _build(ROWS)
    nc = _CACHE["nc"]

    in_maps = [
        {"logits": np.ascontiguousarray(logits[i * ROWS : (i + 1) * ROWS], np.float32)}
        for i in range(NCORES)
    ]
    res = run_bass_kernel_spmd(nc, in_maps, core_ids=list(range(NCORES)), trace=trace)
    LAST_RESULT = res

    s = np.empty(N, np.float32)
    mt = np.empty(N, np.float32)
    for i in range(NCORES):
        s[i * ROWS : (i + 1) * ROWS] = _unpermute(res.results[i]["s"], ROWS)
        mt[i * ROWS : (i + 1) * ROWS] = _unpermute(res.results[i]["mt"], ROWS)

    # mt = exact per-row max (f32); accuracy = logit at the label equals it
    xlab = logits[np.arange(N), labels.astype(np.int64)]
    acc = (xlab == mt).astype(np.float32)
    conf = (np.exp(mt) / s).astype(np.float32)
    return _finish(conf, acc)


# revision 21
# speedup vs baseline: 1.2164x; 1.0592x over previous
"""AdaptiveECE on 8 Trainium2 NeuronCores.

Data-parallel over N=1,000,000 rows: each core streams its 125,000-row
shard of logits [N,128] through SBUF exactly once (the memory-bound part,
64MB/core at ~358GB/s/core HBM) and reduces it to two small per-row
tensors:

  - mt[r] = max_c x[r,c]           (bit-exact fp32 max)
  - s[r]  = sum_c exp(x[r,c])

v3 engine split: the baseline balanced all reduce work across VectorE +
ScalarE (both ~226us busy vs a ~196us DMA stream) because the stock DVE
tensor_reduce streams only 1 element/cycle/lane. v3 replaces it with two
hand-built custom-DVE micro-op programs (per-NEFF opcode_table rows via
concourse's custom-DVE infra):

  SEG_MAX_PAIR / SEG_SUM_PAIR: a 3-state segmented scan. in0/in1 stream
  the two halves of each row (2 elements/cycle through both SBUF read
  ports), the body computes max(a,b) (resp. a+b), a scan stage folds it
  with same-stage CURR_ALU_OUT feedback, a STEP state entered on
  SUB_DIM_DONE resets the accumulator at row boundaries (BYPASS of the
  fresh element), and out_last_subdim_enable writes one result per row.
  One instruction per chunk computes every row's full 128-wide reduce at
  2 elem/cycle - exactly the uop program AWS never wrote for
  tensor_reduce.

Measured: 181-188us on silicon (baseline 249.5us), streaming at a steady
~410 GB/s/core. Per-core busy: DMA ~162us (the sole bottleneck), VectorE
~140us (both segmented reduces), ScalarE ~117us (whole-chunk Exp
big-ops), TensorE/GpSimd idle. Outputs ride the gpsimd SWDGE ring - on
the input HWDGE ring their compute-sem waits head-of-line-block the
stream (worth ~40us). Input DMA count stays low (2x 2MB per chunk):
each DMA_DIRECT2D dispatch occupies the issuing Sync engine ~2-3.7us,
so many small DMAs serialize the stream. The max stays bit-exact fp32,
so the host accuracy test (logits[r, labels[r]] == mt[r]) is unchanged.

The host finishes with O(N) work as the problem's sharding hint
prescribes ("finish ECE on one host"): conf = exp(mt)/s, accuracy, the
global sort of confidences, equal-count bin edges via interp,
searchsorted binning, per-bin (count, conf_sum, acc_sum), and the ECE.

Layout: each partition line holds G=8 consecutive rows (4KB contiguous DMA
runs). Output column (t*G + j), partition p  <->  shard row t*G*128 + p*G + j.
"""

import copy
import sys
import types
from contextlib import ExitStack

import numpy as np

import concourse.bass as bass
import concourse.tile as tile
from concourse import bacc, mybir
from concourse.bass_utils import run_bass_kernel_spmd
import concourse.dve_ops as dve_ops
from concourse.dve_ops import DveOp, OPS
from concourse.dve_spec import Spec, Src0, Src1, AluOp, scan, lower, maxx
from concourse.dve_uop import DveOpSpec, Trigger, AluInp


def _ensure_ntff_hook():
    """bass_utils imports antenv.axon_hooks when tracing is requested; the
    agent image lacks that module. Recreate it (wired to the axon .so) so a
    stray BASS_TRACE=1 in the environment cannot crash the run."""
    try:
        import antenv.axon_hooks  # noqa: F401
        return
    except ImportError:
        pass
    try:
        import antenv
        import trn_agent_boot.trn_boot as tb

        mod = types.ModuleType("antenv.axon_hooks")
        holder = [None]
        mod.set_axon_ntff_profile_hook = lambda h: holder.__setitem__(0, h)
        mod.get_axon_ntff_profile_hook = lambda: holder[0]
        sys.modules["antenv.axon_hooks"] = mod
        antenv.axon_hooks = mod
        try:
            mod.set_axon_ntff_profile_hook(
                tb._ntff_profile_via_ctypes("/opt/axon/libaxon_pjrt.so")
            )
        except Exception:
            pass
    except Exception:
        pass


_ensure_ntff_hook()


# --- custom segmented-reduce DVE ops --------------------------------------

class _SegPairOp(DveOp):
    """Hand-edited custom-DVE op: segmented 2-stream reduce.

    lower() on the (legal) body `scan(op, pair(Src0, Src1))` yields a 2-state
    [seed, steady] program whose scan stage folds with CURR_ALU_OUT feedback
    but never resets. This subclass clones steady into a STEP state that
    BYPASSes the fresh element into the accumulator (= reset to identity,
    inclusive), wires SUB_DIM_DONE transitions like the stock paged-mask
    3-state machine, and write-gates output to the last element of each
    subdim. Result: out[p, s] = reduce over the 2*N elements of row s, at 2
    elements/cycle."""

    def __init__(self, name, spec):
        object.__setattr__(self, "_hand_cache", {})
        super().__init__(name, spec, subdim=True, uops_sha={})

    def compile(self, ver):
        if ver in self._hand_cache:
            return self._hand_cache[ver]
        base = lower(self.spec, ver=ver)
        assert len(base) == 2, f"expected [seed, steady], got {len(base)}"
        seed, steady = copy.deepcopy(base[0]), copy.deepcopy(base[1])
        scan_st = [i for i, dp in enumerate(steady.datapath_config)
                   if dp.alu_src0 == AluInp.CURR_ALU_OUT]
        assert len(scan_st) == 1, scan_st
        step = copy.deepcopy(steady)
        dp = step.datapath_config[scan_st[0]]
        dp.op = AluOp.BYPASS
        dp.alu_src0 = AluInp.PREV_ALU_OUT
        dp.alu_src1 = AluInp.PREV_ALU_OUT
        steady.trigger = (Trigger.SRC_TENSOR_DONE, Trigger.SUB_DIM_DONE, Trigger.NONE)
        steady.next_uop = (0, 2, 0)
        step.trigger = (Trigger.SRC_TENSOR_DONE, Trigger.SUB_DIM_DONE, Trigger.COUNT)
        step.next_uop = (0, 2, 1)
        step.repeat_count = 1
        for u in (steady, step):
            u.out_last_subdim_enable = 1
        r = DveOpSpec(
            name=self.name,
            opcode=dve_ops.get_dve_sub_opcode(self.name),
            uops=[seed, steady, step],
            rd1_en=True,
        )
        self._hand_cache[ver] = r
        return r


def _register(op):
    if op.name in dve_ops._SUB_OPCODE_FOR_NAME:
        return next(o for o in OPS if o.name == op.name)
    OPS.append(op)
    dve_ops._SUB_OPCODE_FOR_NAME[op.name] = (
        dve_ops._CUSTOM_DVE_ROW_BASE + len(OPS) - 1
    )
    dve_ops.CUSTOM_DVE_SPECS[op.name] = op.spec
    return op


def _ref_seg_max(in0, in1, s0, s1, imm2):
    return np.maximum(in0, in1).max(axis=-1)


def _ref_seg_sum(in0, in1, s0, s1, imm2):
    return (in0.astype(np.float32) + in1).sum(axis=-1, dtype=np.float32)


SEG_MAX_PAIR = _register(_SegPairOp(
    "SEG_MAX_PAIR",
    Spec(body=scan(AluOp.MAX, maxx(Src0, Src1)), reference=_ref_seg_max),
))
SEG_SUM_PAIR = _register(_SegPairOp(
    "SEG_SUM_PAIR",
    Spec(body=scan(AluOp.ADD, Src0 + Src1), reference=_ref_seg_sum),
))


# --- kernel ----------------------------------------------------------------

N = 1_000_000
C = 128
NBINS = 15
NCORES = 8
ROWS = N // NCORES  # 125_000 per core
G = 8  # rows per partition line (4KB contiguous DMA runs)
CHUNK_T = 8  # t-groups (of G*128 rows) per chunk

_CACHE: dict = {}
LAST_RESULT = None  # BassKernelResults of the most recent device run


def _build(rows: int, chunk_t: int = CHUNK_T):
    gr = G * 128  # rows per t-group
    tfull = rows // gr  # full t-groups
    tail = rows - tfull * gr  # leftover rows
    tail_p = tail // G  # tail partitions (tail must divide by G)
    assert tail % G == 0, (rows, tail)
    tt = (tfull + (1 if tail else 0)) * G  # output columns

    nc = bacc.Bacc("TRN2", target_bir_lowering=False, debug=False)
    lg = nc.dram_tensor("logits", [rows, C], mybir.dt.float32, kind="ExternalInput").ap()
    s_d = nc.dram_tensor("s", [128, tt], mybir.dt.float32, kind="ExternalOutput").ap()
    mt_d = nc.dram_tensor("mt", [128, tt], mybir.dt.float32, kind="ExternalOutput").ap()

    # [p, t, (j c)] view: row t*1024 + p*8 + j; (j c) is 4KB-contiguous per (p,t)
    lg_t = (
        lg[0 : tfull * gr, :].rearrange("(t p j) c -> p t (j c)", p=128, j=G)
        if tfull
        else None
    )

    h = C // 2  # pair-fold width: in0/in1 stream the two row halves

    with tile.TileContext(nc) as tc, ExitStack() as ctx:
        singles = ctx.enter_context(tc.tile_pool(name="singles", bufs=1))
        xpool = ctx.enter_context(tc.tile_pool(name="x", bufs=3))
        epool = ctx.enter_context(tc.tile_pool(name="e", bufs=2))

        sraw = singles.tile([128, tt], mybir.dt.float32)
        mt_sb = singles.tile([128, tt], mybir.dt.float32)

        # schedule: tail chunk first (its memset off the drain path), a small
        # ramp-in chunk, big chunks, then tapered chunks to shorten the drain
        chunks = []
        t0 = 0
        first = True
        while t0 < tfull:
            left = tfull - t0
            if first:
                n = min(4, left)
                first = False
            elif left > 6:
                n = chunk_t if left > chunk_t else left - 2
            elif left > 4:
                n = left - 2
            else:
                n = left
            chunks.append([t0, n, False])
            t0 += n
        if tail:
            chunks.insert(0, [tfull, 0, True])

        # output DMAs are batched over several chunks: each SWDGE DMA costs a
        # dispatch plus an end-of-kernel semaphore-teardown entry (~90 tiny
        # sem ops cost ~4us of postamble at 34 output DMAs), so fewer+bigger
        # is better. Chunks cover contiguous column ranges, so a group is one
        # contiguous slice. The final group rides the sync ring, idle by then.
        out_group = []

        def _flush_outputs(eng):
            if not out_group:
                return
            glo, ghi = out_group[0][0], out_group[-1][1]
            eng.dma_start(s_d[:, glo:ghi], sraw[:, glo:ghi])
            eng.dma_start(mt_d[:, glo:ghi], mt_sb[:, glo:ghi])
            out_group.clear()

        for t0, nfull, has_tail in chunks:
            nt = nfull + (1 if has_tail else 0)
            ncols = nt * G  # output columns this chunk
            x = xpool.tile([128, ncols, C], mybir.dt.float32)
            for h0, h1 in ((0, nfull // 2), (nfull // 2, nfull)):
                if h1 > h0:
                    nc.sync.dma_start(
                        x[:, h0 * G : h1 * G, :].rearrange(
                            "p a c -> p (a c)"
                        ).rearrange("p (t b) -> p t b", b=G * C),
                        lg_t[:, t0 + h0 : t0 + h1, :],
                    )
            if has_tail:
                nc.vector.memset(x[:, nfull * G :, :], 0.0)
                tail_src = lg[tfull * gr : rows, :].rearrange("(p j) c -> p (j c)", j=G)
                nc.sync.dma_start(
                    x[0:tail_p, nfull * G :, :].rearrange("p a c -> p (a c)"), tail_src
                )

            lo, hi = t0 * G, t0 * G + ncols

            # one VectorE instruction per reduce: 2 elements/cycle, one
            # result per row via the segmented-scan STEP reset. Compute is
            # emitted per DMA half so the first half's ops overlap the second
            # half's transfer and the x buffer frees right after the last
            # half's (short) ops, keeping the input stream fed.
            e = epool.tile([128, ncols, C], mybir.dt.float32)
            bounds = [0, (nfull // 2) * G, ncols] if nfull > 1 else [0, ncols]
            for c0, c1 in zip(bounds[:-1], bounds[1:]):
                if c1 <= c0:
                    continue
                nc.vector._custom_dve(
                    SEG_MAX_PAIR, out=mt_sb[:, lo + c0 : lo + c1],
                    in0=x[:, c0:c1, 0:h], in1=x[:, c0:c1, h:C],
                )
                nc.scalar.activation(
                    e[:, c0:c1, :], x[:, c0:c1, :],
                    mybir.ActivationFunctionType.Exp,
                )
                nc.vector._custom_dve(
                    SEG_SUM_PAIR, out=sraw[:, lo + c0 : lo + c1],
                    in0=e[:, c0:c1, 0:h], in1=e[:, c0:c1, h:C],
                )

            # outputs ride the (otherwise idle) gpsimd SWDGE ring: on the
            # input ring their compute-sem waits would head-of-line-block the
            # next chunk's input stream (HWDGE rings are FIFO per engine)
            out_group.append((lo, hi))
            if has_tail or len(out_group) >= 4:
                _flush_outputs(nc.gpsimd)

        _flush_outputs(nc.sync)

    nc.compile()
    return nc


def _unpermute(a_2d, rows):
    """Device output [128, TT] -> per-row vector [rows].

    Column t*G+j, partition p <-> row t*G*128 + p*G + j.
    """
    gr = G * 128
    tfull = rows // gr
    tail = rows - tfull * gr
    tail_p = tail // G
    out = np.empty(rows, a_2d.dtype)
    nmain = tfull * gr
    out[:nmain] = (
        a_2d[:, : tfull * G].reshape(128, tfull, G).transpose(1, 0, 2).reshape(-1)
    )
    if tail:
        out[nmain:] = a_2d[:tail_p, tfull * G :].reshape(-1)
    return out


def _finish(conf, acc):
    """Mirror of the reference ECE finishing on host."""
    n = conf.shape[0]
    sorted_conf = np.sort(conf)
    q = np.linspace(0.0, float(n), NBINS + 1, dtype=np.float32)
    edges = np.interp(q, np.arange(n, dtype=np.float32), sorted_conf).astype(np.float32)
    idx = np.searchsorted(edges[1:-1], conf, side="left")
    valid = (conf > edges[0]) & (conf <= edges[-1])
    idx = np.where(valid, idx, NBINS)
    cnt = np.bincount(idx, minlength=NBINS + 1)[:NBINS].astype(np.float32)
    csum = np.bincount(idx, weights=conf.astype(np.float64), minlength=NBINS + 1)[
        :NBINS
    ].astype(np.float32)
    asum = np.bincount(idx, weights=acc.astype(np.float64), minlength=NBINS + 1)[
        :NBINS
    ].astype(np.float32)
    prop = cnt / np.float32(n)
    safe = np.maximum(cnt, 1.0)
    gap = np.abs(csum / safe - asum / safe)
    ece = np.sum(np.where(cnt > 0, gap * prop, 0.0), dtype=np.float32)
    return np.asarray(ece, dtype=np.float32).reshape(1)


def kernel(logits, labels, trace: bool = False):
    global LAST_RESULT
    logits = np.asarray(logits)
    labels = np.asarray(labels)
    assert logits.shape == (N, C), logits.shape

    if "nc" not in _CACHE:
        _CACHE["nc"] = _build(ROWS)
    nc = _CACHE["nc"]

    in_maps = [
        {"logits": np.ascontiguousarray(logits[i * ROWS : (i + 1) * ROWS], np.float32)}
        for i in range(NCORES)
    ]
    res = run_bass_kernel_spmd(nc, in_maps, core_ids=list(range(NCORES)), trace=trace)
    LAST_RESULT = res

    s = np.empty(N, np.float32)
    mt = np.empty(N, np.float32)
    for i in range(NCORES):
        s[i * ROWS : (i + 1) * ROWS] = _unpermute(res.results[i]["s"], ROWS)
        mt[i * ROWS : (i + 1) * ROWS] = _unpermute(res.results[i]["mt"], ROWS)

    # mt = exact per-row max (f32); accuracy = logit at the label equals it
    xlab = logits[np.arange(N), labels.astype(np.int64)]
    acc = (xlab == mt).astype(np.float32)
    conf = (np.exp(mt) / s).astype(np.float32)
    return _finish(conf, acc)
